# revision 1
# baseline (speedup 1.0000x reference)
"""Multi-head attention (QKV proj + RoPE + softmax + context) on 8 TRN2 cores.

Problem: B=2, S=2048, DM=2048, H=16, HD=128.
Sharding: tensor-parallel over heads. Core i owns heads (2i, 2i+1); weights are
sliced + transposed on the host, hidden_states is replicated (pre-transposed).

Per-core device program (SPMD, all differences arrive via input data):
  phase A (per batch b): QT/KT/VT projections in [feat, tok] layout via f32r
    matmuls; RoPE applied with a rotation-matrix matmul + fused DVE ops;
    V transposed to [tok, feat] via PE transposes.
  phase B (per b, head): scoresT[k,q] = K @ Q^T tiles; exp via ACT with fused
    1/sqrt(d) scale + additive-mask bias; softmax denominator accumulated on
    DVE and reduced over partitions on GPSIMD; ctxT[d,q] accumulated in PSUM;
    final normalize = ctxT * (1/sum) broadcast.

Output per core: octT[b, h_local, d, s]; host assembles [B, S, DM].
"""
import numpy as np

import concourse.bacc as bacc
import concourse.bass as bass
import concourse.mybir as mybir
import concourse.tile as tile
from concourse.bass_utils import run_bass_kernel_spmd

B, S, DM, H = 2, 2048, 2048, 16
HD = 128                      # head dim
NCORES = 8
HPC = H // NCORES             # heads per core = 2
HFEAT = HPC * HD              # per-core feature slice = 256
NTOK = B * S                  # 4096
KC = DM // 128                # 16 contraction chunks
CT = 512                      # token chunk for projections
NCH = S // CT                 # 4 chunks per batch
KT = S // 128                 # 16 key tiles
QHS = 1024                    # q half size
SCALE = float(1.0 / np.sqrt(HD))

f32 = mybir.dt.float32
f32r = mybir.dt.float32r


def build_program(reps=1):
    nc = bacc.Bacc("TRN2", target_bir_lowering=False, debug=False,
                   num_devices=NCORES)

    hsT = nc.dram_tensor("hsT", [DM, NTOK], f32, kind="ExternalInput").ap()
    wq = nc.dram_tensor("wq", [DM, HFEAT], f32, kind="ExternalInput").ap()
    wk = nc.dram_tensor("wk", [DM, HFEAT], f32, kind="ExternalInput").ap()
    wv = nc.dram_tensor("wv", [DM, HFEAT], f32, kind="ExternalInput").ap()
    bias6 = nc.dram_tensor("bias6", [128, 6], f32, kind="ExternalInput").ap()
    cosT = nc.dram_tensor("cosT", [128, S], f32, kind="ExternalInput").ap()
    sinT = nc.dram_tensor("sinT", [128, S], f32, kind="ExternalInput").ap()
    maskT = nc.dram_tensor("maskT", [128, B * KT], f32, kind="ExternalInput").ap()
    rotT = nc.dram_tensor("rotT", [128, 128], f32, kind="ExternalInput").ap()
    octT = nc.dram_tensor("octT", [B, HPC, HD, S], f32, kind="ExternalOutput").ap()

    with tile.TileContext(nc) as tc:
        import contextlib
        ctx = contextlib.ExitStack()
        with ctx:
            consts = ctx.enter_context(tc.tile_pool(name="consts", bufs=1))
            wpool = ctx.enter_context(tc.tile_pool(name="wpool", bufs=1))
            hspool = ctx.enter_context(tc.tile_pool(name="hspool", bufs=6))
            qkpool = ctx.enter_context(tc.tile_pool(name="qkpool", bufs=1))
            vpool = ctx.enter_context(tc.tile_pool(name="vpool", bufs=1))
            vtpool = ctx.enter_context(tc.tile_pool(name="vtpool", bufs=2))
            tmppool = ctx.enter_context(tc.tile_pool(name="tmppool", bufs=2))
            expool = ctx.enter_context(tc.tile_pool(name="expool", bufs=3))
            accpool = ctx.enter_context(tc.tile_pool(name="accpool", bufs=2))
            normpool = ctx.enter_context(tc.tile_pool(name="normpool", bufs=2))
            outpool = ctx.enter_context(tc.tile_pool(name="outpool", bufs=3))
            ps = ctx.enter_context(tc.tile_pool(name="ps", bufs=4, space="PSUM"))

            # ---- constants (loaded once) ----
            cos_sb = consts.tile([128, S], f32, name="cos_sb")
            sin_sb = consts.tile([128, S], f32, name="sin_sb")
            mask_sb = consts.tile([128, B * KT], f32, name="mask_sb")
            b6_sb = consts.tile([128, 6], f32, name="b6_sb")
            nc.sync.dma_start(out=cos_sb[:], in_=cosT[:])
            nc.sync.dma_start(out=sin_sb[:], in_=sinT[:])
            nc.sync.dma_start(out=mask_sb[:], in_=maskT[:])
            nc.sync.dma_start(out=b6_sb[:], in_=bias6[:])
            rot_sb = consts.tile([128, 128], f32r, name="rot_sb")
            nc.gpsimd.dma_start(out=rot_sb[:], in_=rotT[:])
            ident_sb = consts.tile([128, 128], f32r, name="ident_sb")
            i32 = consts.tile([128, 128], f32, name="i32")
            nc.gpsimd.memset(i32[:], 0.0)
            nc.gpsimd.affine_select(
                out=i32[:], in_=i32[:], compare_op=mybir.AluOpType.not_equal,
                fill=1.0, base=0, pattern=[[-1, 128]], channel_multiplier=1)
            nc.vector.tensor_copy(ident_sb[:], i32[:])

            # ---- weights: [2048, 256] -> [128, kc, 256] f32r via casting DMA
            w_sb = {}
            for nm, src in (("q", wq), ("k", wk), ("v", wv)):
                w_sb[nm] = wpool.tile([128, KC, HFEAT], f32r, name=f"w_{nm}")
                nc.gpsimd.dma_start(
                    out=w_sb[nm][:],
                    in_=src.rearrange("(kc p) n -> p kc n", p=128))

            # per-batch Q/K (rope'd) and V tiles
            qt_sb = [qkpool.tile([128, S], f32r, name=f"qt{h}") for h in range(HPC)]
            kt_sb = [qkpool.tile([128, S], f32r, name=f"kt{h}") for h in range(HPC)]
            v_sb = [vpool.tile([128, KT, HD], f32r, name=f"v{h}") for h in range(HPC)]

            def body():
                for b in range(B):
                    # ======== phase A: projections + rope for batch b ========
                    for c in range(NCH):
                        csl = bass.ts(c, CT)          # token slice within batch
                        q_ps = ps.tile([128, 2 * CT], f32, tag="bank2", name="q_ps")
                        k_ps = ps.tile([128, 2 * CT], f32, tag="bank2", name="k_ps")
                        v_ps = ps.tile([128, 2 * CT], f32, tag="bank2", name="v_ps")
                        for kc in range(KC):
                            hs = hspool.tile([128, CT], f32r, tag="hs", name="hs")
                            nc.gpsimd.dma_start(
                                out=hs[:],
                                in_=hsT[kc * 128:(kc + 1) * 128,
                                        b * S + c * CT: b * S + (c + 1) * CT])
                            st, sp = kc == 0, kc == KC - 1
                            for h in range(HPC):
                                hsl = bass.ts(h, CT)
                                wsl = bass.ts(h, HD)
                                nc.tensor.matmul(q_ps[:, hsl], w_sb["q"][:, kc, wsl],
                                                 hs[:], start=st, stop=sp)
                                nc.tensor.matmul(k_ps[:, hsl], w_sb["k"][:, kc, wsl],
                                                 hs[:], start=st, stop=sp)
                                nc.tensor.matmul(v_ps[:, hsl], w_sb["v"][:, kc, wsl],
                                                 hs[:], start=st, stop=sp)
                        for h in range(HPC):
                            hsl = bass.ts(h, CT)
                            # Q and K: rope
                            for src_ps, bcol, dst in ((q_ps, h, qt_sb[h]),
                                                      (k_ps, 2 + h, kt_sb[h])):
                                half = src_ps[:, hsl]
                                bap = b6_sb[:, bcol:bcol + 1]
                                t1 = tmppool.tile([128, CT], f32, tag="t1", name="t1")
                                nc.vector.scalar_tensor_tensor(
                                    out=t1[:], in0=half, scalar=bap,
                                    in1=cos_sb[:, csl],
                                    op0=mybir.AluOpType.add,
                                    op1=mybir.AluOpType.mult)
                                t0 = tmppool.tile([128, CT], f32r, tag="t0", name="t0")
                                nc.vector.tensor_scalar_add(t0[:], half, bap)
                                r_ps = ps.tile([128, 2 * CT], f32, tag="bank2",
                                               name="r_ps")
                                nc.tensor.matmul(r_ps[:, 0:CT], rot_sb[:], t0[:],
                                                 start=True, stop=True)
                                u = tmppool.tile([128, CT], f32, tag="u", name="u")
                                nc.vector.tensor_mul(u[:], r_ps[:, 0:CT],
                                                     sin_sb[:, csl])
                                nc.vector.tensor_add(dst[:, csl], t1[:], u[:])
                            # V: bias, then transpose into [tok, feat]
                            vt = vtpool.tile([128, CT], f32r, tag="vt", name="vt")
                            nc.vector.tensor_scalar_add(
                                vt[:], v_ps[:, hsl], b6_sb[:, 4 + h:5 + h])
                            tr_ps = ps.tile([128, 2 * CT], f32, tag="bank2",
                                            name="tr_ps")
                            for i in range(4):
                                nc.tensor.transpose(
                                    tr_ps[:, i * 128:(i + 1) * 128].bitcast(f32r),
                                    vt[:, i * 128:(i + 1) * 128], ident_sb[:])
                            for i in range(4):
                                nc.vector.tensor_copy(
                                    v_sb[h][:, c * 4 + i, :],
                                    tr_ps[:, i * 128:(i + 1) * 128].bitcast(f32r))

                    # ======== phase B: attention for batch b ========
                    for h in range(HPC):
                        for qh in range(2):
                            qsl0 = bass.ds(qh * QHS, 512)
                            qsl1 = bass.ds(qh * QHS + 512, 512)
                            ctx_ps = ps.tile([128, QHS], f32, tag="bank2",
                                             name="ctx_ps")
                            acc = accpool.tile([128, QHS], f32, tag="acc",
                                               name="acc")
                            for kt in range(KT):
                                s_ps = ps.tile([128, QHS], f32, tag="bank2",
                                               name="s_ps")
                                ksl = bass.ts(kt, 128)
                                nc.tensor.matmul(s_ps[:, 0:512], kt_sb[h][:, ksl],
                                                 qt_sb[h][:, qsl0],
                                                 start=True, stop=True)
                                nc.tensor.matmul(s_ps[:, 512:1024], kt_sb[h][:, ksl],
                                                 qt_sb[h][:, qsl1],
                                                 start=True, stop=True)
                                ex = expool.tile([128, QHS], f32r, tag="ex",
                                                 name="ex")
                                mcol = b * KT + kt
                                nc.scalar.activation(
                                    ex[:], s_ps[:],
                                    mybir.ActivationFunctionType.Exp,
                                    bias=mask_sb[:, mcol:mcol + 1], scale=SCALE)
                                if kt == 0:
                                    nc.vector.tensor_copy(acc[:],
                                                          ex[:].bitcast(f32))
                                else:
                                    nc.vector.tensor_add(acc[:], acc[:],
                                                         ex[:].bitcast(f32))
                                nc.tensor.matmul(ctx_ps[:, 0:512],
                                                 v_sb[h][:, kt, :], ex[:, 0:512],
                                                 start=(kt == 0), stop=(kt == KT - 1))
                                nc.tensor.matmul(ctx_ps[:, 512:1024],
                                                 v_sb[h][:, kt, :], ex[:, 512:1024],
                                                 start=(kt == 0), stop=(kt == KT - 1))
                            sums = normpool.tile([1, QHS], f32, tag="sums",
                                                 name="sums")
                            nc.gpsimd.tensor_reduce(
                                out=sums[:], in_=acc[:],
                                axis=mybir.AxisListType.C, op=mybir.AluOpType.add)
                            bc = normpool.tile([128, QHS], f32, tag="bc", name="bc")
                            nc.gpsimd.partition_broadcast(bc[:], sums[:])
                            rec = normpool.tile([128, QHS], f32, tag="rec",
                                                name="rec")
                            nc.vector.reciprocal(rec[:], bc[:])
                            out_t = outpool.tile([128, QHS], f32, tag="out",
                                                 name="out_t")
                            nc.vector.tensor_mul(out_t[:], ctx_ps[:], rec[:])
                            nc.sync.dma_start(
                                out=octT[b, h, :, qh * QHS:(qh + 1) * QHS],
                                in_=out_t[:])

            if reps == 1:
                body()
            else:
                with tc.For_i(0, reps, 1):
                    body()
    nc.finalize()
    return nc


_PROGRAM_CACHE = {}


def get_program(reps=1):
    if reps not in _PROGRAM_CACHE:
        _PROGRAM_CACHE[reps] = build_program(reps)
    return _PROGRAM_CACHE[reps]


def make_in_maps(hidden_states, attention_mask, Wq, bq, Wk, bk, Wv, bv):
    hs = np.asarray(hidden_states, dtype=np.float32)
    mask = np.asarray(attention_mask, dtype=np.float32)
    Wq = np.asarray(Wq, dtype=np.float32)
    Wk = np.asarray(Wk, dtype=np.float32)
    Wv = np.asarray(Wv, dtype=np.float32)
    bq = np.asarray(bq, dtype=np.float32)
    bk = np.asarray(bk, dtype=np.float32)
    bv = np.asarray(bv, dtype=np.float32)

    hsT = np.ascontiguousarray(hs.reshape(NTOK, DM).T)

    inv_freq = 1.0 / (10000.0 ** (np.arange(0, HD, 2, dtype=np.float64) / HD))
    t = np.arange(S, dtype=np.float64)
    freqs = t[:, None] * inv_freq[None, :]            # [S, 64]
    emb = np.concatenate([freqs, freqs], axis=1)      # [S, 128]
    cosT = np.ascontiguousarray(np.cos(emb).T).astype(np.float32)
    sinT = np.ascontiguousarray(np.sin(emb).T).astype(np.float32)

    # maskT[kp, b*16+kt] = mask[b, 0, 0, kt*128+kp]
    maskT = np.ascontiguousarray(
        mask.reshape(B, KT, 128).transpose(2, 0, 1).reshape(128, B * KT))

    rot = np.zeros((128, 128), dtype=np.float32)      # lhsT: rot[j,i] = R[i,j]
    for i in range(64):
        rot[i + 64, i] = -1.0                         # R[i, i+64] = -1, i < 64
        rot[i, i + 64] = 1.0                          # R[i+64, i] = +1
    rot = np.ascontiguousarray(rot)

    in_maps = []
    for core in range(NCORES):
        fsl = slice(core * HFEAT, (core + 1) * HFEAT)
        bias6 = np.stack([
            bq[core * HFEAT: core * HFEAT + 128],
            bq[core * HFEAT + 128: core * HFEAT + 256],
            bk[core * HFEAT: core * HFEAT + 128],
            bk[core * HFEAT + 128: core * HFEAT + 256],
            bv[core * HFEAT: core * HFEAT + 128],
            bv[core * HFEAT + 128: core * HFEAT + 256],
        ], axis=1).astype(np.float32)
        in_maps.append({
            "hsT": hsT,
            "wq": np.ascontiguousarray(Wq[fsl, :].T),
            "wk": np.ascontiguousarray(Wk[fsl, :].T),
            "wv": np.ascontiguousarray(Wv[fsl, :].T),
            "bias6": np.ascontiguousarray(bias6),
            "cosT": cosT,
            "sinT": sinT,
            "maskT": maskT,
            "rotT": rot,
        })
    return in_maps


def assemble(results):
    out = np.empty((B, S, DM), dtype=np.float32)
    for core, r in enumerate(results):
        oc = r["octT"]                                # [B, HPC, HD, S]
        for b in range(B):
            for h in range(HPC):
                f0 = (core * HPC + h) * HD
                out[b, :, f0:f0 + HD] = oc[b, h].T
    return out


def kernel(**inputs):
    nc = get_program(reps=1)
    in_maps = make_in_maps(**inputs)
    res = run_bass_kernel_spmd(nc, in_maps, list(range(NCORES)))
    return assemble(res.results)
